# revision 6
# baseline (speedup 1.0000x reference)
"""Segment mean-pool (BERT lattice embedding) Trainium2 Bass kernel.

Full-input contract: kernel(hidden[64,512,768] f32, word_ids[64,512] i32,
num_tokens=400) -> [64,400,768] f32.

Strategy: data-parallel over batch across 8 NeuronCores (8 samples each).
Per sample b the ragged segment mean  out[t] = mean_{s: wid[s]==t} hidden[s]
is computed as a matmul on the PE array with the reciprocal counts folded
into the one-hot matrix (so the matmul directly produces means):

    A_T[s, t] = (word_ids[b, s] == t) * (1 / count[b, word_ids[b, s]])
    psum[t, :] = sum_j A_T[j-chunk].T @ hidden[b, j-chunk]
    out[t, h] = psum[t, h]            pure PSUM->SBUF cast-copy drain

This kernel is memory-bound (22.4 MB f32 I/O per core at ~358 GB/s HBM), so
everything heavy moves in fp16: hidden is cast host-side to fp16 (halves the
input stream), the one-hot is built in fp16, the matmul runs fp16 (full PE
rate, vs fp32r's fp32_mode=HIGH slow path), and the output is written fp16
and upcast host-side. End-to-end error ~4e-4.

Work pruning: word_ids are sorted per sample, so piece-chunk j (128 pieces)
only overlaps a narrow word range. Each (j-chunk, word-chunk) matmul whose
ranges don't intersect (across ALL samples -- the program is shared SPMD)
contributes zeros and is skipped; one-hots are only built over each j's
256-word window. The incidence is measured from the actual word_ids at call
time and the program is compiled for it (7 of 16 pairs for uniform data),
cached per incidence pattern. Word chunks are padded to 4x128 (words
400..511 compare as never-equal -> zeros) so every matmul has full 128-wide
stationary weights.

DMA layout (the critical resource -- one sync-HWDGE ring carries all 11.1
MB at line rate with zero idle gaps): inputs are host-packed p-major so
every descriptor is a 6 KB contiguous run; outputs go to a p-major scratch
[b, p, m, h] (4.6 KB descriptors, ~1.3x the rate of the naive row-major
1.5 KB descriptors) that the host de-transposes during the fp32 upcast; the
400-word tail chunks of all 8 samples batch into one final DMA from the
persistent om tile. Engine split: Tensor ~112 matmuls, ACT drains
psum[:, 0:512] in two-chunk paired instructions, DVE drains psum[:,512:768]
and builds the middle one-hots, GPSIMD builds the edge one-hots.
"""

import numpy as np

B, S, H, T = 64, 512, 768, 400
N_CORES = 8
B_LOC = B // N_CORES  # samples per core
P = 128
J = S // P  # contraction chunks per sample
N0 = 512  # psum bank0 cols (ACT drains); bank1 = H - N0 (DVE drains)
NM = 4  # word chunks of 128 (words 400..511 are compare-never-equal padding)
NM_FULL = 3  # word chunks fully covered by real words (0..383)
TAIL = T - NM_FULL * P  # 16 words in the last chunk
WIN = 256  # one-hot window per piece-chunk (covers <= 2 adjacent word chunks)
GPSIMD_J = ()  # gpsimd tensor_scalar measured ~15x slower than DVE: keep off

# (word-chunk -> piece-chunks that can touch it) for sorted uniform word_ids;
# recomputed from the actual inputs at call time.
DEFAULT_M_JS = ((0, 1), (1, 2), (2, 3), (3,))

_CACHED = {}


def _measure_m_js(wid):
    """Which piece-chunks j intersect word-chunk m, across all samples."""
    m_js = []
    for mi in range(NM):
        t0 = mi * P
        js = []
        for j in range(J):
            w = wid[:, j * P : (j + 1) * P]
            if ((w >= t0) & (w < t0 + P)).any():
                js.append(j)
        m_js.append(tuple(js))
    return tuple(m_js)


def _j_windows(m_js):
    """Per piece-chunk one-hot word-window bases (width WIN, 128-aligned)."""
    j_ms = [[mi for mi in range(NM) if j in m_js[mi]] for j in range(J)]
    bases = []
    for j, ms in enumerate(j_ms):
        if not ms:
            bases.append(0)
            continue
        lo, hi = min(ms), max(ms)
        assert (hi - lo + 1) * P <= WIN, f"chunk {j} spans too many word chunks"
        bases.append(lo * P)
    return bases


def build_program(m_js=DEFAULT_M_JS):
    """Build + compile the single-core Bass program (same NEFF on all cores)."""
    import concourse.bass as bass  # noqa: F401
    import concourse.mybir as mybir
    import concourse.tile as tile
    from concourse import bacc

    nc = bacc.Bacc(
        "TRN2",
        target_bir_lowering=False,
        debug=False,
        enable_asserts=False,
        num_devices=N_CORES,
    )
    f32 = mybir.dt.float32
    f16 = mybir.dt.float16
    bf16 = mybir.dt.bfloat16
    Alu = mybir.AluOpType
    jbase = _j_windows(m_js)

    # hidden host-prearranged as [P, B_LOC, J, H] fp16: partition p holds
    # piece s = 128j + p -> 6 KB contiguous per partition per sample.
    hidden_t = nc.dram_tensor(
        "hidden_pbjh", [P, B_LOC, J, H], f16, kind="ExternalInput"
    ).ap()
    # word_ids (fp32, values < 400 exact) and per-piece reciprocal counts
    # packed together: wr[p, b, j] = (wid[b, 128j+p], 1/count[b, wid[b, 128j+p]])
    wr_t = nc.dram_tensor("wr_pbj", [P, B_LOC, J, 2], f32, kind="ExternalInput").ap()
    # p-major output scratch: out1[b, p, m, h] = out[b, 128m + p, h]
    out1_t = nc.dram_tensor(
        "out1", [B_LOC, P, NM_FULL, H], f16, kind="ExternalOutput"
    ).ap()
    # tail words 384..399 of all samples: out2[p, b, h] = out[b, 384 + p, h]
    out2_t = nc.dram_tensor("out2", [TAIL, B_LOC, H], f16, kind="ExternalOutput").ap()

    with tile.TileContext(nc) as tc:
        with tc.tile_pool(name="const", bufs=1) as const_pool, \
             tc.tile_pool(name="hidp", bufs=B_LOC) as hid_pool, \
             tc.tile_pool(name="aTp", bufs=B_LOC) as aT_pool, \
             tc.tile_pool(name="ps0p", bufs=2, space="PSUM") as ps0_pool, \
             tc.tile_pool(name="ps1p", bufs=2, space="PSUM") as ps1_pool:

            # iota over padded words, fp16 (exact for ints < 2048)
            iota_t = const_pool.tile([P, NM * P], f16, name="iota_t")
            nc.gpsimd.iota(
                iota_t,
                pattern=[[1, NM * P]],
                base=0,
                channel_multiplier=0,
                allow_small_or_imprecise_dtypes=True,
            )

            wr_sb = const_pool.tile([P, B_LOC, J, 2], f32, name="wr_sb")
            nc.sync.dma_start(out=wr_sb, in_=wr_t)

            # persistent output staging for the whole shard (48 KB/partition):
            # nothing recycles, so drains never wait on output DMAs
            om = const_pool.tile([P, B_LOC, NM, H], f16, name="om")

            # Prefetch the whole input shard up front (48 KB/partition).
            hids = []
            for b in range(B_LOC):
                hid = hid_pool.tile([P, J, H], f16, name=f"hid{b}", tag="hid")
                if b == 0:
                    for j in range(J):
                        nc.sync.dma_start(out=hid[:, j, :], in_=hidden_t[:, b, j, :])
                else:
                    nc.sync.dma_start(out=hid, in_=hidden_t[:, b])
                hids.append(hid)

            for b in range(B_LOC):
                hid = hids[b]
                # windowed one-hot * recip, fused in one pass per chunk;
                # edge chunks on gpsimd, middle chunks on DVE
                aT = aT_pool.tile([P, J, WIN], bf16, name="aT", tag="aT")
                for j in range(J):
                    eng = nc.gpsimd if j in GPSIMD_J else nc.vector
                    eng.tensor_scalar(
                        aT[:, j, :],
                        iota_t[:, jbase[j] : jbase[j] + WIN],
                        wr_sb[:, b, j, 0:1],
                        wr_sb[:, b, j, 1:2],
                        op0=Alu.is_equal,
                        op1=Alu.mult,
                    )

                for pair in ((0, 1), (2, 3)):
                    ps0 = ps0_pool.tile([P, 2, N0], f32, name="ps0", tag="ps0")
                    ps1 = ps1_pool.tile([P, 2, H - N0], f32, name="ps1", tag="ps1")
                    for q, mi in enumerate(pair):
                        t0 = mi * P
                        js = m_js[mi]
                        if not js:  # no pieces can hit this word range: zeros
                            nc.vector.memset(om[:, b, mi, :], 0.0)
                            continue
                        for k, j in enumerate(js):
                            st, sp = (k == 0), (k == len(js) - 1)
                            w0 = t0 - jbase[j]
                            wts = aT[:, j, w0 : w0 + P]
                            # back-to-back matmuls share the stationary operand
                            nc.tensor.matmul(
                                ps0[:, q, :], wts, hid[:, j, 0:N0], start=st, stop=sp
                            )
                            nc.tensor.matmul(
                                ps1[:, q, :], wts, hid[:, j, N0:H], start=st, stop=sp
                            )
                    # paired drain: ACT takes bank0 cols, DVE takes bank1 cols
                    m0 = pair[0]
                    nc.scalar.copy(om[:, b, m0 : m0 + 2, 0:N0], ps0)
                    nc.vector.tensor_scalar(
                        om[:, b, m0 : m0 + 2, N0:H], ps1, 0.0, None, op0=Alu.add
                    )

                # full word-chunks stream out per sample, 4.6 KB descriptors
                nc.sync.dma_start(out=out1_t[b], in_=om[:, b, 0:NM_FULL, :])

            # tail words of all samples in one batched DMA
            nc.sync.dma_start(out=out2_t, in_=om[:TAIL, :, NM_FULL, :])

    nc.compile()
    return nc


def _prep_in_maps(hidden, word_ids):
    hidden = np.ascontiguousarray(np.asarray(hidden), dtype=np.float32).reshape(B, S, H)
    wid = np.ascontiguousarray(np.asarray(word_ids), dtype=np.int32).reshape(B, S)

    # per-piece reciprocal counts rp[b, s] = 1/count[b, wid[b, s]]
    counts = np.zeros((B, T), np.int64)
    np.add.at(counts, (np.repeat(np.arange(B), S), wid.reshape(-1)), 1)
    rp = (1.0 / np.maximum(counts, 1))[np.arange(B)[:, None], wid].astype(np.float32)

    h4 = hidden.reshape(B, J, P, H)  # piece s = 128j + p
    w4 = wid.reshape(B, J, P)
    r4 = rp.reshape(B, J, P)

    in_maps = []
    for i in range(N_CORES):
        sl = slice(i * B_LOC, (i + 1) * B_LOC)
        # [B_LOC, J, P, *] -> [P, B_LOC, J, *]
        hs = np.ascontiguousarray(h4[sl].transpose(2, 0, 1, 3).astype(np.float16))
        wr = np.ascontiguousarray(
            np.stack(
                [w4[sl].astype(np.float32), r4[sl]], axis=-1
            ).transpose(2, 0, 1, 3)
        )
        in_maps.append({"hidden_pbjh": hs, "wr_pbj": wr})
    return in_maps


def _assemble_out(results):
    """Invert the p-major scratch layouts and upcast to fp32."""
    outs = []
    for i in range(N_CORES):
        o1 = np.asarray(results[i]["out1"])  # [B_LOC, P, 3, H]
        o2 = np.asarray(results[i]["out2"])  # [TAIL, B_LOC, H]
        full = o1.transpose(0, 2, 1, 3).reshape(B_LOC, NM_FULL * P, H)
        tail = o2.transpose(1, 0, 2)
        outs.append(
            np.concatenate(
                [full.astype(np.float32), tail.astype(np.float32)], axis=1
            )
        )
    return np.concatenate(outs, axis=0)


def run(hidden, word_ids, trace=False, **trace_kwargs):
    from concourse import bass_utils

    m_js = _measure_m_js(
        np.ascontiguousarray(np.asarray(word_ids), dtype=np.int32).reshape(B, S)
    )
    if m_js not in _CACHED:
        _CACHED[m_js] = build_program(m_js)
    nc = _CACHED[m_js]
    in_maps = _prep_in_maps(hidden, word_ids)
    res = bass_utils.run_bass_kernel_spmd(
        nc, in_maps, core_ids=list(range(N_CORES)), trace=trace, **trace_kwargs
    )
    return _assemble_out(res.results), res


def kernel(hidden, word_ids, num_tokens=None, **_unused):
    out, _ = run(hidden, word_ids, trace=False)
    return out


# revision 7
# speedup vs baseline: 1.1908x; 1.1908x over previous
"""Segment mean-pool (BERT lattice embedding) Trainium2 Bass kernel.

Full-input contract: kernel(hidden[64,512,768] f32, word_ids[64,512] i32,
num_tokens=400) -> [64,400,768] f32.

Strategy: data-parallel over batch across 8 NeuronCores (8 samples each).
Per sample b the ragged segment mean  out[t] = mean_{s: wid[s]==t} hidden[s]
is computed as a matmul on the PE array with the reciprocal counts folded
into the one-hot matrix (so the matmul directly produces means):

    A_T[s, t] = (word_ids[b, s] == t) * (1 / count[b, word_ids[b, s]])
    psum[t, :] = sum_j A_T[j-chunk].T @ hidden[b, j-chunk]
    out[t, h] = psum[t, h]            pure PSUM->SBUF cast-copy drain

This kernel is memory-bound (22.4 MB f32 I/O per core at ~358 GB/s HBM), so
everything heavy moves in fp16: hidden is cast host-side to fp16 (halves the
input stream), the one-hot is built in fp16, the matmul runs fp16 (full PE
rate, vs fp32r's fp32_mode=HIGH slow path), and the output is written fp16
and upcast host-side. End-to-end error ~4e-4.

Work pruning: word_ids are sorted per sample, so piece-chunk j (128 pieces)
only overlaps a narrow word range. Each (j-chunk, word-chunk) matmul whose
ranges don't intersect (across ALL samples -- the program is shared SPMD)
contributes zeros and is skipped; one-hots are only built over each j's
256-word window. The incidence is measured from the actual word_ids at call
time and the program is compiled for it (7 of 16 pairs for uniform data),
cached per incidence pattern. Word chunks are padded to 4x128 (words
400..511 compare as never-equal -> zeros) so every matmul has full 128-wide
stationary weights.

DMA layout (the critical resource -- one sync-HWDGE ring carries all 11.1
MB at line rate with zero idle gaps): inputs are host-packed p-major so
every descriptor is a 6 KB contiguous run; outputs go to a p-major scratch
[b, p, m, h] (4.6 KB descriptors, ~1.3x the rate of the naive row-major
1.5 KB descriptors) that the host de-transposes during the fp32 upcast; the
400-word tail chunks of all 8 samples batch into one final DMA from the
persistent om tile. Engine split: Tensor ~112 matmuls, ACT drains
psum[:, 0:512] in two-chunk paired instructions, DVE drains psum[:,512:768]
and builds the middle one-hots, GPSIMD builds the edge one-hots.
"""

import numpy as np

B, S, H, T = 64, 512, 768, 400
N_CORES = 8
B_LOC = B // N_CORES  # samples per core
P = 128
J = S // P  # contraction chunks per sample
N0 = 512  # psum bank0 cols (ACT drains); bank1 = H - N0 (DVE drains)
NM = 4  # word chunks of 128 (words 400..511 are compare-never-equal padding)
NM_FULL = 3  # word chunks fully covered by real words (0..383)
TAIL = T - NM_FULL * P  # 16 words in the last chunk
WIN = 256  # one-hot window per piece-chunk (covers <= 2 adjacent word chunks)
GPSIMD_J = ()  # gpsimd tensor_scalar measured ~15x slower than DVE: keep off

# (word-chunk -> piece-chunks that can touch it) for sorted uniform word_ids;
# recomputed from the actual inputs at call time.
DEFAULT_M_JS = ((0, 1), (1, 2), (2, 3), (3,))

_CACHED = {}


def _measure_m_js(wid):
    """Which piece-chunks j intersect word-chunk m, across all samples."""
    m_js = []
    for mi in range(NM):
        t0 = mi * P
        js = []
        for j in range(J):
            w = wid[:, j * P : (j + 1) * P]
            if ((w >= t0) & (w < t0 + P)).any():
                js.append(j)
        m_js.append(tuple(js))
    return tuple(m_js)


def _j_windows(m_js):
    """Per piece-chunk one-hot word-window bases (width WIN, 128-aligned)."""
    j_ms = [[mi for mi in range(NM) if j in m_js[mi]] for j in range(J)]
    bases = []
    for j, ms in enumerate(j_ms):
        if not ms:
            bases.append(0)
            continue
        lo, hi = min(ms), max(ms)
        assert (hi - lo + 1) * P <= WIN, f"chunk {j} spans too many word chunks"
        bases.append(lo * P)
    return bases


def build_program(m_js=DEFAULT_M_JS):
    """Build + compile the single-core Bass program (same NEFF on all cores)."""
    import concourse.bass as bass  # noqa: F401
    import concourse.mybir as mybir
    import concourse.tile as tile
    from concourse import bacc

    nc = bacc.Bacc(
        "TRN2",
        target_bir_lowering=False,
        debug=False,
        enable_asserts=False,
        num_devices=N_CORES,
    )
    f32 = mybir.dt.float32
    f16 = mybir.dt.float16
    bf16 = mybir.dt.bfloat16
    Alu = mybir.AluOpType
    jbase = _j_windows(m_js)

    # hidden host-prearranged as [P, B_LOC, J, H] fp16: partition p holds
    # piece s = 128j + p -> 6 KB contiguous per partition per sample.
    hidden_t = nc.dram_tensor(
        "hidden_pbjh", [P, B_LOC, J, H], f16, kind="ExternalInput"
    ).ap()
    # word_ids (fp32, values < 400 exact) and per-piece reciprocal counts
    # packed together: wr[p, b, j] = (wid[b, 128j+p], 1/count[b, wid[b, 128j+p]])
    wr_t = nc.dram_tensor("wr_pbj", [P, B_LOC, J, 2], f32, kind="ExternalInput").ap()
    # p-major output scratch: out1[b, p, m, h] = out[b, 128m + p, h]
    out1_t = nc.dram_tensor(
        "out1", [B_LOC, P, NM_FULL, H], f16, kind="ExternalOutput"
    ).ap()
    # tail words 384..399 of all samples: out2[p, b, h] = out[b, 384 + p, h]
    out2_t = nc.dram_tensor("out2", [TAIL, B_LOC, H], f16, kind="ExternalOutput").ap()

    with tile.TileContext(nc) as tc:
        with tc.tile_pool(name="const", bufs=1) as const_pool, \
             tc.tile_pool(name="hidp", bufs=B_LOC) as hid_pool, \
             tc.tile_pool(name="aTp", bufs=B_LOC) as aT_pool, \
             tc.tile_pool(name="ps0p", bufs=2, space="PSUM") as ps0_pool, \
             tc.tile_pool(name="ps1p", bufs=2, space="PSUM") as ps1_pool:

            # iota over padded words, fp16 (exact for ints < 2048)
            iota_t = const_pool.tile([P, NM * P], f16, name="iota_t")
            nc.gpsimd.iota(
                iota_t,
                pattern=[[1, NM * P]],
                base=0,
                channel_multiplier=0,
                allow_small_or_imprecise_dtypes=True,
            )

            wr_sb = const_pool.tile([P, B_LOC, J, 2], f32, name="wr_sb")
            nc.sync.dma_start(out=wr_sb, in_=wr_t)

            # persistent output staging for the whole shard (48 KB/partition):
            # nothing recycles, so drains never wait on output DMAs
            om = const_pool.tile([P, B_LOC, NM, H], f16, name="om")

            # Prefetch the whole input shard up front (48 KB/partition).
            hids = []
            for b in range(B_LOC):
                hid = hid_pool.tile([P, J, H], f16, name=f"hid{b}", tag="hid")
                if b == 0:
                    for j in range(J):
                        nc.sync.dma_start(out=hid[:, j, :], in_=hidden_t[:, b, j, :])
                else:
                    nc.sync.dma_start(out=hid, in_=hidden_t[:, b])
                hids.append(hid)

            for b in range(B_LOC):
                hid = hids[b]
                # windowed one-hot * recip, fused in one pass per chunk;
                # edge chunks on gpsimd, middle chunks on DVE
                aT = aT_pool.tile([P, J, WIN], f16, name="aT", tag="aT")
                for j in range(J):
                    eng = nc.gpsimd if j in GPSIMD_J else nc.vector
                    eng.tensor_scalar(
                        aT[:, j, :],
                        iota_t[:, jbase[j] : jbase[j] + WIN],
                        wr_sb[:, b, j, 0:1],
                        wr_sb[:, b, j, 1:2],
                        op0=Alu.is_equal,
                        op1=Alu.mult,
                    )

                for pair in ((0, 1), (2, 3)):
                    ps0 = ps0_pool.tile([P, 2, N0], f32, name="ps0", tag="ps0")
                    ps1 = ps1_pool.tile([P, 2, H - N0], f32, name="ps1", tag="ps1")
                    for q, mi in enumerate(pair):
                        t0 = mi * P
                        js = m_js[mi]
                        if not js:  # no pieces can hit this word range: zeros
                            nc.vector.memset(om[:, b, mi, :], 0.0)
                            continue
                        for k, j in enumerate(js):
                            st, sp = (k == 0), (k == len(js) - 1)
                            w0 = t0 - jbase[j]
                            wts = aT[:, j, w0 : w0 + P]
                            # back-to-back matmuls share the stationary operand
                            nc.tensor.matmul(
                                ps0[:, q, :], wts, hid[:, j, 0:N0], start=st, stop=sp
                            )
                            nc.tensor.matmul(
                                ps1[:, q, :], wts, hid[:, j, N0:H], start=st, stop=sp
                            )
                    # paired drain: ACT takes bank0 cols, DVE takes bank1 cols
                    m0 = pair[0]
                    nc.scalar.copy(om[:, b, m0 : m0 + 2, 0:N0], ps0)
                    nc.vector.tensor_scalar(
                        om[:, b, m0 : m0 + 2, N0:H], ps1, 0.0, None, op0=Alu.add
                    )

                # full word-chunks stream out per sample, 4.6 KB descriptors
                nc.sync.dma_start(out=out1_t[b], in_=om[:, b, 0:NM_FULL, :])

            # tail words of all samples in one batched DMA
            nc.sync.dma_start(out=out2_t, in_=om[:TAIL, :, NM_FULL, :])

    nc.compile()
    return nc


def _prep_in_maps(hidden, word_ids):
    hidden = np.ascontiguousarray(np.asarray(hidden), dtype=np.float32).reshape(B, S, H)
    wid = np.ascontiguousarray(np.asarray(word_ids), dtype=np.int32).reshape(B, S)

    # per-piece reciprocal counts rp[b, s] = 1/count[b, wid[b, s]]
    counts = np.zeros((B, T), np.int64)
    np.add.at(counts, (np.repeat(np.arange(B), S), wid.reshape(-1)), 1)
    rp = (1.0 / np.maximum(counts, 1))[np.arange(B)[:, None], wid].astype(np.float32)

    h4 = hidden.reshape(B, J, P, H)  # piece s = 128j + p
    w4 = wid.reshape(B, J, P)
    r4 = rp.reshape(B, J, P)

    in_maps = []
    for i in range(N_CORES):
        sl = slice(i * B_LOC, (i + 1) * B_LOC)
        # [B_LOC, J, P, *] -> [P, B_LOC, J, *]
        hs = np.ascontiguousarray(h4[sl].transpose(2, 0, 1, 3).astype(np.float16))
        wr = np.ascontiguousarray(
            np.stack(
                [w4[sl].astype(np.float32), r4[sl]], axis=-1
            ).transpose(2, 0, 1, 3)
        )
        in_maps.append({"hidden_pbjh": hs, "wr_pbj": wr})
    return in_maps


def _assemble_out(results):
    """Invert the p-major scratch layouts and upcast to fp32."""
    outs = []
    for i in range(N_CORES):
        o1 = np.asarray(results[i]["out1"])  # [B_LOC, P, 3, H]
        o2 = np.asarray(results[i]["out2"])  # [TAIL, B_LOC, H]
        full = o1.transpose(0, 2, 1, 3).reshape(B_LOC, NM_FULL * P, H)
        tail = o2.transpose(1, 0, 2)
        outs.append(
            np.concatenate(
                [full.astype(np.float32), tail.astype(np.float32)], axis=1
            )
        )
    return np.concatenate(outs, axis=0)


def run(hidden, word_ids, trace=False, **trace_kwargs):
    from concourse import bass_utils

    m_js = _measure_m_js(
        np.ascontiguousarray(np.asarray(word_ids), dtype=np.int32).reshape(B, S)
    )
    if m_js not in _CACHED:
        _CACHED[m_js] = build_program(m_js)
    nc = _CACHED[m_js]
    in_maps = _prep_in_maps(hidden, word_ids)
    res = bass_utils.run_bass_kernel_spmd(
        nc, in_maps, core_ids=list(range(N_CORES)), trace=trace, **trace_kwargs
    )
    return _assemble_out(res.results), res


def kernel(hidden, word_ids, num_tokens=None, **_unused):
    out, _ = run(hidden, word_ids, trace=False)
    return out
